# revision 6
# baseline (speedup 1.0000x reference)
"""Trainium2 Bass kernel for the ragged-graph actor/critic model.

Contract: kernel(**inputs) takes the FULL (unsharded) inputs as produced by
setup_inputs() and returns the full outputs, matching the reference:
    (node_states [N,256] f32, actions [B] i32, log_prob [B,1] f32,
     ent [B,1] f32, v [B,1] f32)

Strategy (8 NeuronCores, SPMD):
  - Launch A — data-parallel over graphs (contiguous graph ranges balanced by
    node count). Host marshals feature-major (transposed) node/goal slabs and
    replicated weights. Per core: stream 512-node column blocks; fp32r
    matmuls on PE compute ns^T[hid,nodes] = W_in-chunks.T @ x^T, ReLU+bias on
    ACT, then the actor+critic heads (packed [320,32] weight, then a
    block-diagonal [32,2]) producing per-node logits and critic values.
  - Host reshards: ragged [N] logits/crit -> dense [B, 768] (pure indexing),
    128 graphs per core.
  - Launch B — per-graph math on [128 graphs, 768] tiles: masked softmax
    log-sum-exp, entropy, gumbel-max sampling (same noise as
    jax.random.categorical(key(42), ...)), critic max+mean. Fully static.
"""

import sys
import os

for _p in ("/opt/trn_rl_repo", "/root/.axon_site/_ro/trn_rl_repo"):
    if os.path.isdir(_p) and _p not in sys.path:
        sys.path.insert(0, _p)

import numpy as np

import concourse.bass as bass
import concourse.tile as tile
import concourse.mybir as mybir
from concourse.bass_utils import run_bass_kernel_spmd

F32 = mybir.dt.float32
F32R = mybir.dt.float32r
I32 = mybir.dt.int32
Alu = mybir.AluOpType
Act = mybir.ActivationFunctionType
AX = mybir.AxisListType

NEG_BIG = -1.0e30


class Cfg:
    def __init__(self, n_cores=8, node_cap=66560, graphs_per_core=128,
                 jblk=512, feat=256, hid=256, goal=64, w_d=768, bigc=1024.0):
        assert node_cap % jblk == 0
        self.n_cores = n_cores
        self.node_cap = node_cap
        self.gpcore = graphs_per_core
        self.jblk = jblk
        self.feat = feat
        self.hid = hid
        self.goal = goal
        self.w_d = w_d          # dense per-graph width (>= max graph size)
        self.bigc = bigc        # > w_d; used for argmax index trick


DEFAULT_CFG = Cfg()


# --------------------------------------------------------------------------
# Post-pass: walrus here accepts at most one sem wait per instruction.
# --------------------------------------------------------------------------

def _split_syncs(nc):
    ctr = 0
    for f in nc.m.functions:
        for bb in f.blocks:
            out = []
            changed = False
            for inst in list(bb.instructions):
                si = inst.sync_info
                nw = len(si.on_wait) if si and si.on_wait else 0
                if nw > 1:
                    waits = list(si.on_wait)
                    for w in waits[:-1]:
                        ctr += 1
                        nop = mybir.InstNoOp(name=f"SWN-{ctr}", ins=[], outs=[])
                        nop.engine = inst.engine
                        nop.sync_info = mybir.SyncInfo(on_wait=[w], on_update=[])
                        out.append(nop)
                    si.on_wait = [waits[-1]]
                    changed = True
                out.append(inst)
            if changed:
                bb.instructions = out
    return nc


# --------------------------------------------------------------------------
# Launch A: input MLP + heads over packed node rows
# --------------------------------------------------------------------------

def build_program_a(cfg: Cfg):
    c = cfg
    nblk = c.node_cap // c.jblk

    nc = bass.Bass("TRN2", target_bir_lowering=False, debug=False,
                   num_devices=c.n_cores)

    x_t = nc.dram_tensor("x_t", [c.feat, c.node_cap], F32R, kind="ExternalInput")
    g_t = nc.dram_tensor("g_t", [c.goal, c.node_cap], F32R, kind="ExternalInput")
    w_in = nc.dram_tensor("w_in", [c.feat, c.hid], F32R, kind="ExternalInput")
    w_ac = nc.dram_tensor("w_ac", [c.hid + c.goal, 32], F32R, kind="ExternalInput")
    w2bd = nc.dram_tensor("w2bd", [32, 2], F32R, kind="ExternalInput")
    b_hid = nc.dram_tensor("b_hid", [c.hid, 1], F32, kind="ExternalInput")
    b_ac = nc.dram_tensor("b_ac", [32, 1], F32, kind="ExternalInput")

    ns_t = nc.dram_tensor("ns_t", [c.hid, c.node_cap], F32R, kind="ExternalOutput")
    pk = nc.dram_tensor("pk", [2, c.node_cap], F32, kind="ExternalOutput")

    kh = c.hid // 128
    kf = c.feat // 128

    with tile.TileContext(nc) as tc:
        with (
            tc.tile_pool(name="consts", bufs=1) as cpool,
            tc.tile_pool(name="xin", bufs=3) as xpool,
            tc.tile_pool(name="gin", bufs=3) as gpool,
            tc.tile_pool(name="ns", bufs=3) as nspool,
            tc.tile_pool(name="h1", bufs=2) as hpool,
            tc.tile_pool(name="lc", bufs=2) as lcpool,
            tc.tile_pool(name="psum", bufs=2, space="PSUM") as ppool,
        ):
            w_sb = []
            for k in range(kf):
                t = cpool.tile([128, c.hid], F32R, tag=f"w_in{k}")
                nc.sync.dma_start(t[:], w_in[k * 128:(k + 1) * 128, :])
                w_sb.append(t)
            wac_sb = []
            ac_chunks = []
            pos = 0
            while pos < c.hid + c.goal:
                sz = min(128, c.hid + c.goal - pos)
                t = cpool.tile([sz, 32], F32R, tag=f"w_ac{pos}")
                nc.sync.dma_start(t[:], w_ac[pos:pos + sz, :])
                wac_sb.append(t)
                ac_chunks.append((pos, sz))
                pos += sz
            w2_sb = cpool.tile([32, 2], F32R, tag="w2bd")
            nc.sync.dma_start(w2_sb[:], w2bd[:])
            bh_sb = []
            for k in range(kh):
                t = cpool.tile([128, 1], F32, tag=f"b_hid{k}")
                nc.sync.dma_start(t[:], b_hid[k * 128:(k + 1) * 128, :])
                bh_sb.append(t)
            bac_sb = cpool.tile([32, 1], F32, tag="b_ac")
            nc.sync.dma_start(bac_sb[:], b_ac[:])

            J = c.jblk
            for j in range(nblk):
                sl = slice(j * J, (j + 1) * J)
                xs = []
                for k in range(kf):
                    xt = xpool.tile([128, J], F32R, tag=f"x{k}")
                    nc.sync.dma_start(xt[:], x_t[k * 128:(k + 1) * 128, sl])
                    xs.append(xt)
                gt = gpool.tile([c.goal, J], F32R, tag="g")
                nc.sync.dma_start(gt[:], g_t[:, sl])

                ns_sb = []
                for m in range(kh):
                    ph = ppool.tile([128, J], F32, tag=f"ph{m}")
                    for k in range(kf):
                        nc.tensor.matmul(
                            ph[:],
                            w_sb[k][:, m * 128:(m + 1) * 128],
                            xs[k][:],
                            start=(k == 0), stop=(k == kf - 1),
                        )
                    nst = nspool.tile([128, J], F32R, tag=f"ns{m}")
                    nc.scalar.activation(nst[:], ph[:], Act.Relu,
                                         bias=bh_sb[m][:])
                    nc.sync.dma_start(ns_t[m * 128:(m + 1) * 128, sl], nst[:])
                    ns_sb.append(nst)

                pac = ppool.tile([32, J], F32, tag="pac")
                n_ac = len(wac_sb)
                for i, (pos, sz) in enumerate(ac_chunks):
                    rhs = ns_sb[pos // 128][:] if pos < c.hid else gt[:]
                    nc.tensor.matmul(pac[:], wac_sb[i][:],
                                     rhs,
                                     start=(i == 0), stop=(i == n_ac - 1))
                h1 = hpool.tile([32, J], F32R, tag="h1")
                nc.scalar.activation(h1[:], pac[:], Act.Relu, bias=bac_sb[:])

                plc = ppool.tile([2, J], F32, tag="plc")
                nc.tensor.matmul(plc[:], w2_sb[:],
                                 h1[:], start=True, stop=True)
                lc = lcpool.tile([2, J], F32, tag="lc")
                nc.vector.tensor_copy(lc[:], plc[:])
                nc.sync.dma_start(pk[0:2, sl], lc[:])

    _split_syncs(nc)
    return nc


# --------------------------------------------------------------------------
# Launch B: per-graph masked softmax / entropy / gumbel-argmax / critic agg
# --------------------------------------------------------------------------

def build_program_b(cfg: Cfg):
    c = cfg
    G = c.gpcore
    W = c.w_d
    assert G <= 128

    nc = bass.Bass("TRN2", target_bir_lowering=False, debug=False,
                   num_devices=c.n_cores)

    dd = nc.dram_tensor("dd", [G, W], F32, kind="ExternalInput")    # logits dense
    cd = nc.dram_tensor("cd", [G, W], F32, kind="ExternalInput")    # crit dense
    maskA = nc.dram_tensor("maskA", [G, W], F32, kind="ExternalInput")
    gumB = nc.dram_tensor("gumB", [G, W], F32, kind="ExternalInput")
    m01 = nc.dram_tensor("m01", [G, W], F32, kind="ExternalInput")
    pgc = nc.dram_tensor("pgc", [G, 2], F32, kind="ExternalInput")
    iotar = nc.dram_tensor("iotar", [G, W], F32, kind="ExternalInput")

    vout = nc.dram_tensor("vout", [1, G], F32, kind="ExternalOutput")
    entout = nc.dram_tensor("entout", [1, G], F32, kind="ExternalOutput")
    lpout = nc.dram_tensor("lpout", [1, G], F32, kind="ExternalOutput")
    actout = nc.dram_tensor("actout", [1, G], I32, kind="ExternalOutput")

    with tile.TileContext(nc) as tc:
        with tc.tile_pool(name="pb", bufs=1) as bp:
            D = bp.tile([G, W], F32, tag="D")
            nc.sync.dma_start(D[:], dd[:])
            C = bp.tile([G, W], F32, tag="C")
            nc.sync.dma_start(C[:], cd[:])
            At = bp.tile([G, W], F32, tag="At")
            nc.sync.dma_start(At[:], maskA[:])
            BGt = bp.tile([G, W], F32, tag="BGt")
            nc.sync.dma_start(BGt[:], gumB[:])
            M01t = bp.tile([G, W], F32, tag="M01t")
            nc.sync.dma_start(M01t[:], m01[:])
            pgct = bp.tile([G, 2], F32, tag="pgct")
            nc.sync.dma_start(pgct[:], pgc[:])
            iot = bp.tile([G, W], F32, tag="iot")
            nc.sync.dma_start(iot[:], iotar[:])
            invnn = pgct[:, 0:1]
            bc2b = pgct[:, 1:2]

            # ---- critic: v = 0.5*segmax + 0.5*segsum/nn + bc2 ----
            Cm = bp.tile([G, W], F32, tag="Cm")
            nc.vector.tensor_tensor(Cm[:], C[:], At[:], Alu.add)
            segmax = bp.tile([G, 1], F32, tag="segmax")
            nc.vector.reduce_max(segmax[:], Cm[:], axis=AX.X)
            scr = bp.tile([G, W], F32, tag="scr")
            segsum = bp.tile([G, 1], F32, tag="segsum")
            nc.vector.scalar_tensor_tensor(scr[:], C[:], 1.0, M01t[:],
                                           Alu.mult, Alu.mult,
                                           accum_out=segsum[:])
            t1 = bp.tile([G, 1], F32, tag="t1")
            nc.vector.tensor_scalar(out=t1[:], in0=segsum[:], scalar1=invnn,
                                    scalar2=None, op0=Alu.mult)
            vraw = bp.tile([G, 1], F32, tag="vraw")
            nc.vector.tensor_tensor(vraw[:], t1[:], segmax[:], Alu.add)
            vfin = bp.tile([G, 1], F32, tag="vfin")
            nc.scalar.activation(vfin[:], vraw[:], Act.Identity,
                                 bias=bc2b, scale=0.5)
            nc.sync.dma_start(vout[0:1, :], vfin[:])

            # ---- actor: log-softmax pieces ----
            Dm = bp.tile([G, W], F32, tag="Dm")
            nc.vector.tensor_tensor(Dm[:], D[:], At[:], Alu.add)
            M = bp.tile([G, 1], F32, tag="M")
            nc.vector.reduce_max(M[:], Dm[:], axis=AX.X)
            negM = bp.tile([G, 1], F32, tag="negM")
            nc.vector.tensor_scalar_mul(negM[:], M[:], -1.0)
            E = bp.tile([G, W], F32, tag="E")
            S = bp.tile([G, 1], F32, tag="S")
            nc.scalar.activation(E[:], Dm[:], Act.Exp, bias=negM[:],
                                 scale=1.0, accum_out=S[:])
            scr2 = bp.tile([G, W], F32, tag="scr2")
            sumT = bp.tile([G, 1], F32, tag="sumT")
            nc.vector.scalar_tensor_tensor(scr2[:], Dm[:], 1.0, E[:],
                                           Alu.mult, Alu.mult,
                                           accum_out=sumT[:])
            logS = bp.tile([G, 1], F32, tag="logS")
            nc.scalar.activation(logS[:], S[:], Act.Ln)
            lse = bp.tile([G, 1], F32, tag="lse")
            nc.vector.tensor_tensor(lse[:], M[:], logS[:], Alu.add)
            negS = bp.tile([G, 1], F32, tag="negS")
            nc.vector.tensor_scalar_mul(negS[:], S[:], -1.0)
            nrS = bp.tile([G, 1], F32, tag="nrS")
            nc.vector.reciprocal(nrS[:], negS[:])
            ent = bp.tile([G, 1], F32, tag="ent")
            nc.vector.scalar_tensor_tensor(ent[:], sumT[:], nrS, lse[:],
                                           Alu.mult, Alu.add)
            nc.sync.dma_start(entout[0:1, :], ent[:])

            # ---- actions: argmax(D + gumbel + mask) ----
            Z = bp.tile([G, W], F32, tag="Z")
            nc.vector.tensor_tensor(Z[:], D[:], BGt[:], Alu.add)
            zmax = bp.tile([G, 1], F32, tag="zmax")
            nc.vector.reduce_max(zmax[:], Z[:], axis=AX.X)
            tk = bp.tile([G, W], F32, tag="tk")
            nc.vector.scalar_tensor_tensor(tk[:], Z[:], zmax, iot[:],
                                           Alu.is_equal, Alu.mult)
            tkmax = bp.tile([G, 1], F32, tag="tkmax")
            nc.vector.reduce_max(tkmax[:], tk[:], axis=AX.X)
            actf = bp.tile([G, 1], F32, tag="actf")
            nc.vector.tensor_scalar(out=actf[:], in0=tkmax[:], scalar1=-1.0,
                                    scalar2=c.bigc, op0=Alu.mult, op1=Alu.add)
            acti = bp.tile([G, 1], I32, tag="acti")
            nc.vector.tensor_copy(acti[:], actf[:])
            nc.sync.dma_start(actout[0:1, :], acti[:])

            # ---- log_prob at sampled action ----
            scr3 = bp.tile([G, W], F32, tag="scr3")
            picked = bp.tile([G, 1], F32, tag="picked")
            nc.vector.scalar_tensor_tensor(scr3[:], iot[:], tkmax, Dm[:],
                                           Alu.is_equal, Alu.mult,
                                           accum_out=picked[:])
            lp = bp.tile([G, 1], F32, tag="lp")
            nc.vector.tensor_tensor(lp[:], picked[:], lse[:], Alu.subtract)
            nc.sync.dma_start(lpout[0:1, :], lp[:])

    _split_syncs(nc)
    return nc


# --------------------------------------------------------------------------
# Host marshalling
# --------------------------------------------------------------------------

def _gumbel(B, max_n):
    import jax
    import jax.numpy as jnp
    try:
        cpu = jax.devices("cpu")[0]
        with jax.default_device(cpu):
            g = jax.random.gumbel(jax.random.key(42), (B, max_n), jnp.float32)
            return np.asarray(g)
    except Exception:
        g = jax.random.gumbel(jax.random.key(42), (B, max_n), jnp.float32)
        return np.asarray(g)


def _split(off, n_total, n_cores):
    bounds = [0]
    for k in range(1, n_cores):
        target = k * n_total / n_cores
        g = int(np.argmin(np.abs(np.asarray(off, dtype=np.float64) - target)))
        g = max(g, bounds[-1] + 1)
        bounds.append(g)
    bounds.append(len(off) - 1)
    return bounds


_PROGRAM_CACHE = {}


def run(cfg, inputs, goal, num_nodes, W_in, b_in, Wa1, ba1, Wa2, ba2,
        Wc1, bc1, Wc2, bc2, trace=False, timers=None):
    import time as _time
    c = cfg
    x = np.ascontiguousarray(np.asarray(inputs, np.float32))
    gl = np.ascontiguousarray(np.asarray(goal, np.float32))
    sizes = np.asarray(num_nodes).astype(np.int64)
    B = sizes.shape[0]
    N = x.shape[0]
    off = np.concatenate([[0], np.cumsum(sizes)])
    assert off[-1] == N, f"sum(num_nodes)={off[-1]} != N={N}"
    assert sizes.min() >= 1 and sizes.max() <= c.w_d
    assert B == c.n_cores * c.gpcore

    bounds = _split(off, N, c.n_cores)

    w_ac = np.concatenate([np.asarray(Wa1, np.float32),
                           np.asarray(Wc1, np.float32)], axis=1)
    w2bd = np.zeros((32, 2), np.float32)
    w2bd[0:16, 0] = np.asarray(Wa2, np.float32).ravel()
    w2bd[16:32, 1] = np.asarray(Wc2, np.float32).ravel()
    b_hid = np.asarray(b_in, np.float32).reshape(c.hid, 1)
    b_ac = np.concatenate([np.asarray(ba1, np.float32),
                           np.asarray(bc1, np.float32)]).reshape(32, 1)
    bc2v = float(np.asarray(bc2).ravel()[0])

    xt_full = np.ascontiguousarray(x.T)      # [feat, N]
    gt_full = np.ascontiguousarray(gl.T)     # [goal, N]

    in_maps_a = []
    meta = []
    for ci in range(c.n_cores):
        g0, g1 = bounds[ci], bounds[ci + 1]
        n0, n1 = int(off[g0]), int(off[g1])
        ncnt = n1 - n0
        assert ncnt <= c.node_cap, (ci, ncnt, c.node_cap)
        x_t = np.zeros((c.feat, c.node_cap), np.float32)
        x_t[:, :ncnt] = xt_full[:, n0:n1]
        g_t = np.zeros((c.goal, c.node_cap), np.float32)
        g_t[:, :ncnt] = gt_full[:, n0:n1]
        in_maps_a.append({
            "x_t": x_t, "g_t": g_t,
            "w_in": np.asarray(W_in, np.float32),
            "w_ac": w_ac, "w2bd": w2bd, "b_hid": b_hid, "b_ac": b_ac,
        })
        meta.append((g0, g1, n0, ncnt))

    key_a = ("A", c.n_cores, c.node_cap, c.jblk)
    if key_a not in _PROGRAM_CACHE:
        _PROGRAM_CACHE[key_a] = build_program_a(c)
    nca = _PROGRAM_CACHE[key_a]
    t0 = _time.time()
    res_a = run_bass_kernel_spmd(nca, in_maps_a, list(range(c.n_cores)),
                                 trace=trace)
    t1 = _time.time()

    # ---- host reshard: packed per-node -> dense per-graph ----
    logits = np.empty(N, np.float32)
    crit = np.empty(N, np.float32)
    for ci, (g0, g1, n0, ncnt) in enumerate(meta):
        pkc = res_a.results[ci]["pk"]
        logits[n0:n0 + ncnt] = pkc[0, :ncnt]
        crit[n0:n0 + ncnt] = pkc[1, :ncnt]

    W = c.w_d
    j = np.arange(W)[None, :]
    idx = np.minimum(off[:-1, None] + j, N - 1)
    valid = j < sizes[:, None]
    dd = logits[idx]
    cdn = crit[idx]
    gum = _gumbel(B, W)
    maskA = np.where(valid, 0.0, NEG_BIG).astype(np.float32)
    gumB = (gum + maskA).astype(np.float32)
    m01 = valid.astype(np.float32)
    pgc = np.empty((B, 2), np.float32)
    pgc[:, 0] = 1.0 / sizes
    pgc[:, 1] = bc2v
    iotar = np.broadcast_to(
        (c.bigc - np.arange(W, dtype=np.float32))[None, :], (c.gpcore, W))
    iotar = np.ascontiguousarray(iotar)

    G = c.gpcore
    in_maps_b = []
    for ci in range(c.n_cores):
        s = slice(ci * G, (ci + 1) * G)
        in_maps_b.append({
            "dd": np.ascontiguousarray(dd[s]),
            "cd": np.ascontiguousarray(cdn[s]),
            "maskA": maskA[s], "gumB": gumB[s], "m01": m01[s],
            "pgc": pgc[s], "iotar": iotar,
        })

    key_b = ("B", c.n_cores, c.gpcore, c.w_d)
    if key_b not in _PROGRAM_CACHE:
        _PROGRAM_CACHE[key_b] = build_program_b(c)
    ncb = _PROGRAM_CACHE[key_b]
    t2 = _time.time()
    res_b = run_bass_kernel_spmd(ncb, in_maps_b, list(range(c.n_cores)),
                                 trace=trace)
    t3 = _time.time()
    if timers is not None:
        timers["launch_a_s"] = t1 - t0
        timers["launch_b_s"] = t3 - t2

    # ---- unshard ----
    node_states = np.empty((N, c.hid), np.float32)
    for ci, (g0, g1, n0, ncnt) in enumerate(meta):
        node_states[n0:n0 + ncnt, :] = res_a.results[ci]["ns_t"][:, :ncnt].T
    actions = np.concatenate(
        [res_b.results[ci]["actout"].reshape(-1) for ci in range(c.n_cores)])
    log_prob = np.concatenate(
        [res_b.results[ci]["lpout"].reshape(-1) for ci in range(c.n_cores)])
    ent = np.concatenate(
        [res_b.results[ci]["entout"].reshape(-1) for ci in range(c.n_cores)])
    v = np.concatenate(
        [res_b.results[ci]["vout"].reshape(-1) for ci in range(c.n_cores)])
    return (node_states, actions.astype(np.int32), log_prob[:, None],
            ent[:, None], v[:, None])


def kernel(inputs, goal, num_nodes, W_in, b_in, Wa1, ba1, Wa2, ba2,
           Wc1, bc1, Wc2, bc2):
    return run(DEFAULT_CFG, inputs, goal, num_nodes, W_in, b_in,
               Wa1, ba1, Wa2, ba2, Wc1, bc1, Wc2, bc2)


# revision 7
# speedup vs baseline: 85539.9985x; 85539.9985x over previous
"""Trainium2 Bass kernel for the ragged-graph actor/critic model.

Contract: kernel(**inputs) takes the FULL (unsharded) inputs as produced by
setup_inputs() and returns the full outputs, matching the reference:
    (node_states [N,256] f32, actions [B] i32, log_prob [B,1] f32,
     ent [B,1] f32, v [B,1] f32)

Strategy (8 NeuronCores, SPMD):
  - Launch A — data-parallel over graphs (contiguous graph ranges balanced by
    node count). Host marshals feature-major (transposed) node/goal slabs and
    replicated weights. Per core: stream 512-node column blocks; fp32r
    matmuls on PE compute ns^T[hid,nodes] = W_in-chunks.T @ x^T, ReLU+bias on
    ACT, then the actor+critic heads (packed [320,32] weight, then a
    block-diagonal [32,2]) producing per-node logits and critic values.
  - Host reshards: ragged [N] logits/crit -> dense [B, 768] (pure indexing),
    128 graphs per core.
  - Launch B — per-graph math on [128 graphs, 768] tiles: masked softmax
    log-sum-exp, entropy, gumbel-max sampling (same noise as
    jax.random.categorical(key(42), ...)), critic max+mean. Fully static.
"""

import sys
import os

for _p in ("/opt/trn_rl_repo", "/root/.axon_site/_ro/trn_rl_repo"):
    if os.path.isdir(_p) and _p not in sys.path:
        sys.path.insert(0, _p)

import numpy as np

import concourse.bass as bass
import concourse.tile as tile
import concourse.mybir as mybir
from concourse.bass_utils import run_bass_kernel_spmd

F32 = mybir.dt.float32
F32R = mybir.dt.float32r
I32 = mybir.dt.int32
Alu = mybir.AluOpType
Act = mybir.ActivationFunctionType
AX = mybir.AxisListType

NEG_BIG = -1.0e30


class Cfg:
    def __init__(self, n_cores=8, node_cap=66560, graphs_per_core=128,
                 jblk=512, feat=256, hid=256, goal=64, w_d=768, bigc=1024.0):
        assert node_cap % jblk == 0
        self.n_cores = n_cores
        self.node_cap = node_cap
        self.gpcore = graphs_per_core
        self.jblk = jblk
        self.feat = feat
        self.hid = hid
        self.goal = goal
        self.w_d = w_d          # dense per-graph width (>= max graph size)
        self.bigc = bigc        # > w_d; used for argmax index trick


DEFAULT_CFG = Cfg()


# --------------------------------------------------------------------------
# Post-pass: walrus here accepts at most one sem wait per instruction.
# --------------------------------------------------------------------------

def _split_syncs(nc):
    ctr = 0
    for f in nc.m.functions:
        for bb in f.blocks:
            out = []
            changed = False
            for inst in list(bb.instructions):
                si = inst.sync_info
                nw = len(si.on_wait) if si and si.on_wait else 0
                if nw > 1:
                    waits = list(si.on_wait)
                    for w in waits[:-1]:
                        ctr += 1
                        nop = mybir.InstNoOp(name=f"SWN-{ctr}", ins=[], outs=[])
                        nop.engine = inst.engine
                        nop.sync_info = mybir.SyncInfo(on_wait=[w], on_update=[])
                        out.append(nop)
                    si.on_wait = [waits[-1]]
                    changed = True
                out.append(inst)
            if changed:
                bb.instructions = out
    return nc


# --------------------------------------------------------------------------
# Launch A: input MLP + heads over packed node rows
# --------------------------------------------------------------------------

def build_program_a(cfg: Cfg):
    c = cfg
    nblk = c.node_cap // c.jblk

    nc = bass.Bass("TRN2", target_bir_lowering=False, debug=False,
                   num_devices=c.n_cores)

    x_t = nc.dram_tensor("x_t", [c.feat, c.node_cap], F32R, kind="ExternalInput")
    g_t = nc.dram_tensor("g_t", [c.goal, c.node_cap], F32R, kind="ExternalInput")
    w_in = nc.dram_tensor("w_in", [c.feat, c.hid], F32R, kind="ExternalInput")
    w_ac = nc.dram_tensor("w_ac", [c.hid + c.goal, 32], F32R, kind="ExternalInput")
    w2bd = nc.dram_tensor("w2bd", [32, 2], F32R, kind="ExternalInput")
    b_hid = nc.dram_tensor("b_hid", [c.hid, 1], F32, kind="ExternalInput")
    b_ac = nc.dram_tensor("b_ac", [32, 1], F32, kind="ExternalInput")

    ns_t = nc.dram_tensor("ns_t", [c.hid, c.node_cap], F32R, kind="ExternalOutput")
    pk = nc.dram_tensor("pk", [2, c.node_cap], F32, kind="ExternalOutput")

    kh = c.hid // 128
    kf = c.feat // 128

    with tile.TileContext(nc) as tc:
        with (
            tc.tile_pool(name="consts", bufs=1) as cpool,
            tc.tile_pool(name="xin", bufs=3) as xpool,
            tc.tile_pool(name="gin", bufs=3) as gpool,
            tc.tile_pool(name="ns", bufs=3) as nspool,
            tc.tile_pool(name="h1", bufs=2) as hpool,
            tc.tile_pool(name="lc", bufs=2) as lcpool,
            tc.tile_pool(name="psum", bufs=2, space="PSUM") as ppool,
        ):
            w_sb = []
            for k in range(kf):
                t = cpool.tile([128, c.hid], F32R, tag=f"w_in{k}")
                nc.sync.dma_start(t[:], w_in[k * 128:(k + 1) * 128, :])
                w_sb.append(t)
            wac_sb = []
            ac_chunks = []
            pos = 0
            while pos < c.hid + c.goal:
                sz = min(128, c.hid + c.goal - pos)
                t = cpool.tile([sz, 32], F32R, tag=f"w_ac{pos}")
                nc.sync.dma_start(t[:], w_ac[pos:pos + sz, :])
                wac_sb.append(t)
                ac_chunks.append((pos, sz))
                pos += sz
            w2_sb = cpool.tile([32, 2], F32R, tag="w2bd")
            nc.sync.dma_start(w2_sb[:], w2bd[:])
            bh_sb = []
            for k in range(kh):
                t = cpool.tile([128, 1], F32, tag=f"b_hid{k}")
                nc.sync.dma_start(t[:], b_hid[k * 128:(k + 1) * 128, :])
                bh_sb.append(t)
            bac_sb = cpool.tile([32, 1], F32, tag="b_ac")
            nc.sync.dma_start(bac_sb[:], b_ac[:])

            J = c.jblk
            for j in range(nblk):
                sl = slice(j * J, (j + 1) * J)
                xs = []
                for k in range(kf):
                    xt = xpool.tile([128, J], F32R, tag=f"x{k}")
                    nc.gpsimd.dma_start(xt[:], x_t[k * 128:(k + 1) * 128, sl])
                    xs.append(xt)
                gt = gpool.tile([c.goal, J], F32R, tag="g")
                nc.gpsimd.dma_start(gt[:], g_t[:, sl])

                ns_sb = []
                for m in range(kh):
                    ph = ppool.tile([128, J], F32, tag=f"ph{m}")
                    for k in range(kf):
                        nc.tensor.matmul(
                            ph[:],
                            w_sb[k][:, m * 128:(m + 1) * 128],
                            xs[k][:],
                            start=(k == 0), stop=(k == kf - 1),
                        )
                    nst = nspool.tile([128, J], F32R, tag=f"ns{m}")
                    nc.scalar.activation(nst[:], ph[:], Act.Relu,
                                         bias=bh_sb[m][:])
                    nc.sync.dma_start(ns_t[m * 128:(m + 1) * 128, sl], nst[:])
                    ns_sb.append(nst)

                pac = ppool.tile([32, J], F32, tag="pac")
                n_ac = len(wac_sb)
                for i, (pos, sz) in enumerate(ac_chunks):
                    rhs = ns_sb[pos // 128][:] if pos < c.hid else gt[:]
                    nc.tensor.matmul(pac[:], wac_sb[i][:],
                                     rhs,
                                     start=(i == 0), stop=(i == n_ac - 1))
                h1 = hpool.tile([32, J], F32R, tag="h1")
                nc.scalar.activation(h1[:], pac[:], Act.Relu, bias=bac_sb[:])

                plc = ppool.tile([2, J], F32, tag="plc")
                nc.tensor.matmul(plc[:], w2_sb[:],
                                 h1[:], start=True, stop=True)
                lc = lcpool.tile([2, J], F32, tag="lc")
                nc.vector.tensor_copy(lc[:], plc[:])
                nc.sync.dma_start(pk[0:2, sl], lc[:])

    _split_syncs(nc)
    return nc


# --------------------------------------------------------------------------
# Launch B: per-graph masked softmax / entropy / gumbel-argmax / critic agg
# --------------------------------------------------------------------------

def build_program_b(cfg: Cfg):
    c = cfg
    G = c.gpcore
    W = c.w_d
    assert G <= 128

    nc = bass.Bass("TRN2", target_bir_lowering=False, debug=False,
                   num_devices=c.n_cores)

    dd = nc.dram_tensor("dd", [G, W], F32, kind="ExternalInput")    # logits dense
    cd = nc.dram_tensor("cd", [G, W], F32, kind="ExternalInput")    # crit dense
    maskA = nc.dram_tensor("maskA", [G, W], F32, kind="ExternalInput")
    gumB = nc.dram_tensor("gumB", [G, W], F32, kind="ExternalInput")
    m01 = nc.dram_tensor("m01", [G, W], F32, kind="ExternalInput")
    pgc = nc.dram_tensor("pgc", [G, 2], F32, kind="ExternalInput")
    iotar = nc.dram_tensor("iotar", [G, W], F32, kind="ExternalInput")

    vout = nc.dram_tensor("vout", [1, G], F32, kind="ExternalOutput")
    entout = nc.dram_tensor("entout", [1, G], F32, kind="ExternalOutput")
    lpout = nc.dram_tensor("lpout", [1, G], F32, kind="ExternalOutput")
    actout = nc.dram_tensor("actout", [1, G], I32, kind="ExternalOutput")

    with tile.TileContext(nc) as tc:
        with tc.tile_pool(name="pb", bufs=1) as bp:
            D = bp.tile([G, W], F32, tag="D")
            nc.sync.dma_start(D[:], dd[:])
            C = bp.tile([G, W], F32, tag="C")
            nc.sync.dma_start(C[:], cd[:])
            At = bp.tile([G, W], F32, tag="At")
            nc.sync.dma_start(At[:], maskA[:])
            BGt = bp.tile([G, W], F32, tag="BGt")
            nc.sync.dma_start(BGt[:], gumB[:])
            M01t = bp.tile([G, W], F32, tag="M01t")
            nc.sync.dma_start(M01t[:], m01[:])
            pgct = bp.tile([G, 2], F32, tag="pgct")
            nc.sync.dma_start(pgct[:], pgc[:])
            iot = bp.tile([G, W], F32, tag="iot")
            nc.sync.dma_start(iot[:], iotar[:])
            invnn = pgct[:, 0:1]
            bc2b = pgct[:, 1:2]

            # ---- critic: v = 0.5*segmax + 0.5*segsum/nn + bc2 ----
            Cm = bp.tile([G, W], F32, tag="Cm")
            nc.vector.tensor_tensor(Cm[:], C[:], At[:], Alu.add)
            segmax = bp.tile([G, 1], F32, tag="segmax")
            nc.vector.reduce_max(segmax[:], Cm[:], axis=AX.X)
            scr = bp.tile([G, W], F32, tag="scr")
            segsum = bp.tile([G, 1], F32, tag="segsum")
            nc.vector.scalar_tensor_tensor(scr[:], C[:], 1.0, M01t[:],
                                           Alu.mult, Alu.mult,
                                           accum_out=segsum[:])
            t1 = bp.tile([G, 1], F32, tag="t1")
            nc.vector.tensor_scalar(out=t1[:], in0=segsum[:], scalar1=invnn,
                                    scalar2=None, op0=Alu.mult)
            vraw = bp.tile([G, 1], F32, tag="vraw")
            nc.vector.tensor_tensor(vraw[:], t1[:], segmax[:], Alu.add)
            vfin = bp.tile([G, 1], F32, tag="vfin")
            nc.scalar.activation(vfin[:], vraw[:], Act.Identity,
                                 bias=bc2b, scale=0.5)
            nc.sync.dma_start(vout[0:1, :], vfin[:])

            # ---- actor: log-softmax pieces ----
            Dm = bp.tile([G, W], F32, tag="Dm")
            nc.vector.tensor_tensor(Dm[:], D[:], At[:], Alu.add)
            M = bp.tile([G, 1], F32, tag="M")
            nc.vector.reduce_max(M[:], Dm[:], axis=AX.X)
            negM = bp.tile([G, 1], F32, tag="negM")
            nc.vector.tensor_scalar_mul(negM[:], M[:], -1.0)
            E = bp.tile([G, W], F32, tag="E")
            S = bp.tile([G, 1], F32, tag="S")
            nc.scalar.activation(E[:], Dm[:], Act.Exp, bias=negM[:],
                                 scale=1.0, accum_out=S[:])
            scr2 = bp.tile([G, W], F32, tag="scr2")
            sumT = bp.tile([G, 1], F32, tag="sumT")
            nc.vector.scalar_tensor_tensor(scr2[:], Dm[:], 1.0, E[:],
                                           Alu.mult, Alu.mult,
                                           accum_out=sumT[:])
            logS = bp.tile([G, 1], F32, tag="logS")
            nc.scalar.activation(logS[:], S[:], Act.Ln)
            lse = bp.tile([G, 1], F32, tag="lse")
            nc.vector.tensor_tensor(lse[:], M[:], logS[:], Alu.add)
            negS = bp.tile([G, 1], F32, tag="negS")
            nc.vector.tensor_scalar_mul(negS[:], S[:], -1.0)
            nrS = bp.tile([G, 1], F32, tag="nrS")
            nc.vector.reciprocal(nrS[:], negS[:])
            ent = bp.tile([G, 1], F32, tag="ent")
            nc.vector.scalar_tensor_tensor(ent[:], sumT[:], nrS, lse[:],
                                           Alu.mult, Alu.add)
            nc.sync.dma_start(entout[0:1, :], ent[:])

            # ---- actions: argmax(D + gumbel + mask) ----
            Z = bp.tile([G, W], F32, tag="Z")
            nc.vector.tensor_tensor(Z[:], D[:], BGt[:], Alu.add)
            zmax = bp.tile([G, 1], F32, tag="zmax")
            nc.vector.reduce_max(zmax[:], Z[:], axis=AX.X)
            tk = bp.tile([G, W], F32, tag="tk")
            nc.vector.scalar_tensor_tensor(tk[:], Z[:], zmax, iot[:],
                                           Alu.is_equal, Alu.mult)
            tkmax = bp.tile([G, 1], F32, tag="tkmax")
            nc.vector.reduce_max(tkmax[:], tk[:], axis=AX.X)
            actf = bp.tile([G, 1], F32, tag="actf")
            nc.vector.tensor_scalar(out=actf[:], in0=tkmax[:], scalar1=-1.0,
                                    scalar2=c.bigc, op0=Alu.mult, op1=Alu.add)
            acti = bp.tile([G, 1], I32, tag="acti")
            nc.vector.tensor_copy(acti[:], actf[:])
            nc.sync.dma_start(actout[0:1, :], acti[:])

            # ---- log_prob at sampled action ----
            scr3 = bp.tile([G, W], F32, tag="scr3")
            picked = bp.tile([G, 1], F32, tag="picked")
            nc.vector.scalar_tensor_tensor(scr3[:], iot[:], tkmax, Dm[:],
                                           Alu.is_equal, Alu.mult,
                                           accum_out=picked[:])
            lp = bp.tile([G, 1], F32, tag="lp")
            nc.vector.tensor_tensor(lp[:], picked[:], lse[:], Alu.subtract)
            nc.sync.dma_start(lpout[0:1, :], lp[:])

    _split_syncs(nc)
    return nc


# --------------------------------------------------------------------------
# Host marshalling
# --------------------------------------------------------------------------

def _gumbel(B, max_n):
    import jax
    import jax.numpy as jnp
    try:
        cpu = jax.devices("cpu")[0]
        with jax.default_device(cpu):
            g = jax.random.gumbel(jax.random.key(42), (B, max_n), jnp.float32)
            return np.asarray(g)
    except Exception:
        g = jax.random.gumbel(jax.random.key(42), (B, max_n), jnp.float32)
        return np.asarray(g)


def _split(off, n_total, n_cores):
    bounds = [0]
    for k in range(1, n_cores):
        target = k * n_total / n_cores
        g = int(np.argmin(np.abs(np.asarray(off, dtype=np.float64) - target)))
        g = max(g, bounds[-1] + 1)
        bounds.append(g)
    bounds.append(len(off) - 1)
    return bounds


_PROGRAM_CACHE = {}


def run(cfg, inputs, goal, num_nodes, W_in, b_in, Wa1, ba1, Wa2, ba2,
        Wc1, bc1, Wc2, bc2, trace=False, timers=None):
    import time as _time
    c = cfg
    x = np.ascontiguousarray(np.asarray(inputs, np.float32))
    gl = np.ascontiguousarray(np.asarray(goal, np.float32))
    sizes = np.asarray(num_nodes).astype(np.int64)
    B = sizes.shape[0]
    N = x.shape[0]
    off = np.concatenate([[0], np.cumsum(sizes)])
    assert off[-1] == N, f"sum(num_nodes)={off[-1]} != N={N}"
    assert sizes.min() >= 1 and sizes.max() <= c.w_d
    assert B == c.n_cores * c.gpcore

    bounds = _split(off, N, c.n_cores)

    w_ac = np.concatenate([np.asarray(Wa1, np.float32),
                           np.asarray(Wc1, np.float32)], axis=1)
    w2bd = np.zeros((32, 2), np.float32)
    w2bd[0:16, 0] = np.asarray(Wa2, np.float32).ravel()
    w2bd[16:32, 1] = np.asarray(Wc2, np.float32).ravel()
    b_hid = np.asarray(b_in, np.float32).reshape(c.hid, 1)
    b_ac = np.concatenate([np.asarray(ba1, np.float32),
                           np.asarray(bc1, np.float32)]).reshape(32, 1)
    bc2v = float(np.asarray(bc2).ravel()[0])

    xt_full = np.ascontiguousarray(x.T)      # [feat, N]
    gt_full = np.ascontiguousarray(gl.T)     # [goal, N]

    in_maps_a = []
    meta = []
    for ci in range(c.n_cores):
        g0, g1 = bounds[ci], bounds[ci + 1]
        n0, n1 = int(off[g0]), int(off[g1])
        ncnt = n1 - n0
        assert ncnt <= c.node_cap, (ci, ncnt, c.node_cap)
        x_t = np.zeros((c.feat, c.node_cap), np.float32)
        x_t[:, :ncnt] = xt_full[:, n0:n1]
        g_t = np.zeros((c.goal, c.node_cap), np.float32)
        g_t[:, :ncnt] = gt_full[:, n0:n1]
        in_maps_a.append({
            "x_t": x_t, "g_t": g_t,
            "w_in": np.asarray(W_in, np.float32),
            "w_ac": w_ac, "w2bd": w2bd, "b_hid": b_hid, "b_ac": b_ac,
        })
        meta.append((g0, g1, n0, ncnt))

    key_a = ("A", c.n_cores, c.node_cap, c.jblk)
    if key_a not in _PROGRAM_CACHE:
        _PROGRAM_CACHE[key_a] = build_program_a(c)
    nca = _PROGRAM_CACHE[key_a]
    t0 = _time.time()
    res_a = run_bass_kernel_spmd(nca, in_maps_a, list(range(c.n_cores)),
                                 trace=trace)
    t1 = _time.time()

    # ---- host reshard: packed per-node -> dense per-graph ----
    logits = np.empty(N, np.float32)
    crit = np.empty(N, np.float32)
    for ci, (g0, g1, n0, ncnt) in enumerate(meta):
        pkc = res_a.results[ci]["pk"]
        logits[n0:n0 + ncnt] = pkc[0, :ncnt]
        crit[n0:n0 + ncnt] = pkc[1, :ncnt]

    W = c.w_d
    j = np.arange(W)[None, :]
    idx = np.minimum(off[:-1, None] + j, N - 1)
    valid = j < sizes[:, None]
    dd = logits[idx]
    cdn = crit[idx]
    gum = _gumbel(B, W)
    maskA = np.where(valid, 0.0, NEG_BIG).astype(np.float32)
    gumB = (gum + maskA).astype(np.float32)
    m01 = valid.astype(np.float32)
    pgc = np.empty((B, 2), np.float32)
    pgc[:, 0] = 1.0 / sizes
    pgc[:, 1] = bc2v
    iotar = np.broadcast_to(
        (c.bigc - np.arange(W, dtype=np.float32))[None, :], (c.gpcore, W))
    iotar = np.ascontiguousarray(iotar)

    G = c.gpcore
    in_maps_b = []
    for ci in range(c.n_cores):
        s = slice(ci * G, (ci + 1) * G)
        in_maps_b.append({
            "dd": np.ascontiguousarray(dd[s]),
            "cd": np.ascontiguousarray(cdn[s]),
            "maskA": maskA[s], "gumB": gumB[s], "m01": m01[s],
            "pgc": pgc[s], "iotar": iotar,
        })

    key_b = ("B", c.n_cores, c.gpcore, c.w_d)
    if key_b not in _PROGRAM_CACHE:
        _PROGRAM_CACHE[key_b] = build_program_b(c)
    ncb = _PROGRAM_CACHE[key_b]
    t2 = _time.time()
    res_b = run_bass_kernel_spmd(ncb, in_maps_b, list(range(c.n_cores)),
                                 trace=trace)
    t3 = _time.time()
    if timers is not None:
        timers["launch_a_s"] = t1 - t0
        timers["launch_b_s"] = t3 - t2

    # ---- unshard ----
    node_states = np.empty((N, c.hid), np.float32)
    for ci, (g0, g1, n0, ncnt) in enumerate(meta):
        node_states[n0:n0 + ncnt, :] = res_a.results[ci]["ns_t"][:, :ncnt].T
    actions = np.concatenate(
        [res_b.results[ci]["actout"].reshape(-1) for ci in range(c.n_cores)])
    log_prob = np.concatenate(
        [res_b.results[ci]["lpout"].reshape(-1) for ci in range(c.n_cores)])
    ent = np.concatenate(
        [res_b.results[ci]["entout"].reshape(-1) for ci in range(c.n_cores)])
    v = np.concatenate(
        [res_b.results[ci]["vout"].reshape(-1) for ci in range(c.n_cores)])
    return (node_states, actions.astype(np.int32), log_prob[:, None],
            ent[:, None], v[:, None])


def kernel(inputs, goal, num_nodes, W_in, b_in, Wa1, ba1, Wa2, ba2,
           Wc1, bc1, Wc2, bc2):
    return run(DEFAULT_CFG, inputs, goal, num_nodes, W_in, b_in,
               Wa1, ba1, Wa2, ba2, Wc1, bc1, Wc2, bc2)
